# revision 16
# baseline (speedup 1.0000x reference)
"""Trainium2 Bass kernel for top-1 MoE routing (deepspeed top1gating) + expert FFN.

Strategy (8 NeuronCores):
  Launch 1 (token-parallel gate): core k handles tokens [k*S/8, (k+1)*S/8):
    - host supplies the x slice pre-transposed ([D, Sl] f32), so logits
      l = x @ w_gate come straight off PE (fp32 keeps argmax faithful),
    - per-token: argmax expert id (vector.max/max_index), top-1 softmax prob
      g = 1/sum(exp(l - max)) via ACT Exp with accum_out + DVE reciprocal.
  Launch 2 (expert-parallel routing positions): core k owns expert k. With
    tokens laid out [128, S/128] (s = col*128 + p), computes the reference's
    locations1 = cumsum(mask) - mask via a triangular-matrix matmul
    (within-column prefix), a DVE tensor_tensor_scan over column totals
    (across-column prefix), and a rank-1 matmul broadcast; then the capacity
    keep-mask valid = mask * (pos < C). All routing DECISIONS are on device.
  Host glue (placement only): ids[pos[valid]] = token_id[valid]; gather the
    kept rows of bf16 x (zero row for empty slots) into the transposed
    [D, C]-layout block the matmuls want, and per-slot gate values. The bf16
    cast and all weight layout packing/casting happen on host.
  Launch 3 (expert-parallel FFN): core k owns expert k:
    - hT[f, c] = sum_d w1[d, f] xg[c, d]   (lhsT = w1 packed [128,FC,DC,128])
      gelu (tanh approx, matching jax.nn.gelu default) via x*sigmoid(2c(x+a x^3))
      outT[d, c] = sum_f w2[f, d] hT[f, c]  (lhsT = w2 packed, rhs = hT;
      w2 stationary so each weight load streams 1024 columns, like mm1)
    - scale slot columns by gate value, write compact [D, C] bf16 output.
  Host scatter: out[ids[c]] = out_cT[:, c] for filled slots; dropped and
  unrouted tokens stay zero, exactly like the reference's dense combine.
"""

import functools

import ml_dtypes
import numpy as np

import concourse.bacc as bacc
import concourse.mybir as mybir
import concourse.tile as tile
from concourse.bass_utils import run_bass_kernel_spmd

F32 = mybir.dt.float32
BF16 = mybir.dt.bfloat16
U32 = mybir.dt.uint32

N_CORES = 8


# --------------------------------------------------------------------------
# Launch 1: gate (token-parallel)
# --------------------------------------------------------------------------
def build_gate_nc(S, D, E, n_cores=N_CORES, reps=1):
    """Per core: xT_sl [D, Sl] f32 (host-transposed slice) ->
    e_out [128, T] u32, g_out [128, T] f32.  (token s_local = t*128 + p)"""
    Sl = S // n_cores
    T = Sl // 128
    DC = D // 128
    nc = bacc.Bacc("TRN2", target_bir_lowering=False, debug=False)
    xT = nc.dram_tensor("xT", [D, Sl], F32, kind="ExternalInput")
    wg = nc.dram_tensor("wg", [D, E], F32, kind="ExternalInput")
    e_out = nc.dram_tensor("e_out", [128, T], U32, kind="ExternalOutput")
    g_out = nc.dram_tensor("g_out", [128, T], F32, kind="ExternalOutput")

    xTr = xT.ap().rearrange("(c p) s -> p c s", p=128)

    with tile.TileContext(nc) as tc:
        with (
            tc.tile_pool(name="const", bufs=1) as constp,
            tc.tile_pool(name="xin", bufs=3) as xin,
            tc.tile_pool(name="small", bufs=4) as small,
            tc.tile_pool(name="res", bufs=1) as resp,
            tc.tile_pool(name="psl", bufs=2, space="PSUM") as psl,
        ):
            wg_sb = constp.tile([128, DC, E], F32)
            nc.sync.dma_start(wg_sb[:], wg.ap().rearrange("(c p) e -> p c e", p=128))
            e_sb = resp.tile([128, T], U32)
            g_sb = resp.tile([128, T], F32)

            for t in [t for _ in range(reps) for t in range(T)]:
                xt_sb = xin.tile([128, DC, 128], F32)
                nc.sync.dma_start(xt_sb[:], xTr[:, :, t * 128 : (t + 1) * 128])
                # logits [128 s, E] accumulated over d-chunks
                pl = psl.tile([128, E], F32)
                for d in range(DC):
                    nc.tensor.matmul(
                        pl[:],
                        lhsT=xt_sb[:, d, :],
                        rhs=wg_sb[:, d, :],
                        start=(d == 0),
                        stop=(d == DC - 1),
                    )
                lsb = small.tile([128, E], F32)
                nc.vector.tensor_copy(lsb[:], pl[:])
                # g = 1 / sum(exp(l - max))  (softmax value at the argmax)
                mx = small.tile([128, 1], F32)
                nc.vector.tensor_reduce(
                    mx[:], lsb[:], axis=mybir.AxisListType.X, op=mybir.AluOpType.max
                )
                nmx = small.tile([128, 1], F32)
                nc.vector.tensor_scalar_mul(nmx[:], mx[:], -1.0)
                ex = small.tile([128, E], F32)
                den = small.tile([128, 1], F32)
                nc.scalar.activation(
                    ex[:],
                    lsb[:],
                    mybir.ActivationFunctionType.Exp,
                    bias=nmx[:],
                    scale=1.0,
                    accum_out=den[:],
                )
                nc.vector.reciprocal(g_sb[:, t : t + 1], den[:])
                # argmax over experts (first max wins, like jnp.argmax)
                top8 = small.tile([128, 8], F32)
                nc.vector.max(top8[:], lsb[:])
                midx = small.tile([128, 8], U32)
                nc.vector.max_index(midx[:], top8[:], lsb[:])
                nc.vector.tensor_copy(e_sb[:, t : t + 1], midx[:, 0:1])
                # per-tile result DMA keeps every rep's work observable
                nc.sync.dma_start(e_out.ap()[:, t : t + 1], e_sb[:, t : t + 1])
                nc.sync.dma_start(g_out.ap()[:, t : t + 1], g_sb[:, t : t + 1])
    nc.compile()
    return nc


# --------------------------------------------------------------------------
# Launch 2: routing positions (expert-parallel)
# --------------------------------------------------------------------------
def build_route_nc(S, E, reps=1):
    """Per core (expert k): exclusive position of each token within expert k
    plus the capacity keep-mask.

    Inputs : ef [128, J] f32 (expert id per token; s = j*128 + p),
             eid [128, 1] f32, ut [128, 128] f32 (ut[q, p] = 1 if q <= p).
    Outputs: pos_out [128, J] f32, valid_out [128, J] f32.
    """
    J = S // 128
    C = S // E
    nc = bacc.Bacc("TRN2", target_bir_lowering=False, debug=False)
    ef = nc.dram_tensor("ef", [128, J], F32, kind="ExternalInput")
    eid = nc.dram_tensor("eid", [128, 1], F32, kind="ExternalInput")
    ut = nc.dram_tensor("ut", [128, 128], F32, kind="ExternalInput")
    pos_out = nc.dram_tensor("pos_out", [128, J], F32, kind="ExternalOutput")

    with tile.TileContext(nc) as tc:
        with (
            tc.tile_pool(name="c1", bufs=1) as c1,
            tc.tile_pool(name="p", bufs=4) as p,
            tc.tile_pool(name="ps", bufs=2, space="PSUM") as ps,
            tc.tile_pool(name="pst", bufs=2, space="PSUM") as pst,
        ):
            # constants loaded once per launch (excluded from per-rep slope)
            ut_sb = c1.tile([128, 128], F32)
            nc.sync.dma_start(ut_sb[:], ut.ap())
            eid_sb = c1.tile([128, 1], F32)
            nc.sync.dma_start(eid_sb[:], eid.ap())
            onesc = c1.tile([128, 1], F32)
            nc.vector.memset(onesc[:], 1.0)
            onesr = c1.tile([1, 128], F32)
            nc.vector.memset(onesr[:], 1.0)
            zrow = c1.tile([1, J], F32)
            nc.vector.memset(zrow[:], 0.0)

            for _rep in range(reps):
                ef_sb = p.tile([128, J], F32)
                nc.sync.dma_start(ef_sb[:], ef.ap())
                m_sb = p.tile([128, J], F32)
                nc.vector.tensor_scalar(
                    m_sb[:], ef_sb[:], eid_sb[:], None, op0=mybir.AluOpType.is_equal
                )
                # within-column inclusive prefix over partitions
                ps_incl = ps.tile([128, J], F32)
                nc.tensor.matmul(
                    ps_incl[:], lhsT=ut_sb[:], rhs=m_sb[:], start=True, stop=True
                )
                # column totals via ones-vector matmul (no cross-partition DMA)
                ps_tot = pst.tile([1, J], F32)
                nc.tensor.matmul(
                    ps_tot[:], lhsT=onesc[:], rhs=m_sb[:], start=True, stop=True
                )
                trow = p.tile([1, J], F32)
                nc.vector.tensor_copy(trow[:], ps_tot[:])
                # exclusive prefix across columns (tokens are column-major:
                # s = j*128 + p)
                sc = p.tile([1, J], F32)
                nc.vector.tensor_tensor_scan(
                    sc[:],
                    trow[:],
                    zrow[:],
                    0.0,
                    op0=mybir.AluOpType.add,
                    op1=mybir.AluOpType.add,
                )
                off = p.tile([1, J], F32)
                nc.vector.tensor_sub(off[:], sc[:], trow[:])
                # broadcast offsets over partitions via rank-1 matmul
                ps_off = ps.tile([128, J], F32)
                nc.tensor.matmul(
                    ps_off[:], lhsT=onesr[:], rhs=off[:], start=True, stop=True
                )
                # exclusive position = incl - m + off  (psum read as DVE input)
                pos_sb = p.tile([128, J], F32)
                nc.vector.tensor_sub(pos_sb[:], ps_incl[:], m_sb[:])
                nc.vector.tensor_add(pos_sb[:], pos_sb[:], ps_off[:])
                # fold the keep-mask into the sign: out = (pos+1)*keep - 1,
                # keep = member & (pos < capacity); host reads valid = out >= 0
                v_sb = p.tile([128, J], F32)
                nc.vector.tensor_scalar(
                    v_sb[:], pos_sb[:], float(C), None, op0=mybir.AluOpType.is_lt
                )
                nc.vector.tensor_mul(v_sb[:], v_sb[:], m_sb[:])
                p1_sb = p.tile([128, J], F32)
                nc.vector.tensor_scalar(
                    p1_sb[:], pos_sb[:], 1.0, None, op0=mybir.AluOpType.add
                )
                nc.vector.tensor_mul(p1_sb[:], p1_sb[:], v_sb[:])
                nc.vector.tensor_scalar(
                    p1_sb[:], p1_sb[:], -1.0, None, op0=mybir.AluOpType.add
                )
                nc.sync.dma_start(pos_out.ap()[:], p1_sb[:])
    nc.compile()
    return nc


# --------------------------------------------------------------------------
# Launch 3: expert FFN (expert-parallel)
# --------------------------------------------------------------------------
def build_ffn_nc(S, D, E, F, reps=1):
    """Per core (expert k): gathered tokens -> gelu MLP -> scaled compact out.

    Inputs : xgT [128, DC*C] bf16 (xgT[p, d*C + c] = xg[c, d*128 + p]),
             grow [1, C] f32 (gate value of slot c),
             w1p [128, FC, DC, 128] bf16 (w1p[p,f,c,j] = w1[c*128+p, f*128+j]),
             w2p [128, DC, FC, 128] bf16 (w2p[p,c,f,j] = w2[f*128+p, c*128+j]).
    Outputs: out_cT [D, C] f32 (out_cT[d, c] = gate-scaled expert out, slot c).
    """
    C = S // E
    DC = D // 128
    FC = F // 128
    NB = min(512, C)
    FG = 8  # w2 f-chunks fetched per DMA

    nc = bacc.Bacc("TRN2", target_bir_lowering=False, debug=False)
    xgT_in = nc.dram_tensor("xgT", [128, DC * C], BF16, kind="ExternalInput")
    grow_in = nc.dram_tensor("grow", [1, C], F32, kind="ExternalInput")
    w1p = nc.dram_tensor("w1p", [128, FC, DC, 128], BF16, kind="ExternalInput")
    w2p = nc.dram_tensor("w2p", [128, DC, FC, 128], BF16, kind="ExternalInput")
    out_cT = nc.dram_tensor("out_cT", [D, C], BF16, kind="ExternalOutput")

    with tile.TileContext(nc) as tc:
      for _rep in range(reps):
        with (
            tc.tile_pool(name="hTp", bufs=1) as hTp,
            tc.tile_pool(name="ggp", bufs=1) as ggp,
        ):
            hT = hTp.tile([128, FC, C], BF16)
            # broadcast per-slot gate over partitions via rank-1 matmul
            grow = ggp.tile([1, C], F32)
            nc.sync.dma_start(grow[:], grow_in.ap())
            onesr = ggp.tile([1, 128], F32)
            nc.vector.memset(onesr[:], 1.0)
            gbc = ggp.tile([128, C], F32)
            with tc.tile_pool(name="psg", bufs=1, space="PSUM") as psg:
                pg = psg.tile([128, C], F32)
                for h in range(0, C, NB):
                    nc.tensor.matmul(
                        pg[:, h : h + NB],
                        lhsT=onesr[:],
                        rhs=grow[:, h : h + NB],
                        start=True,
                        stop=True,
                    )
                nc.vector.tensor_copy(gbc[:], pg[:])
            with tc.tile_pool(name="xgp", bufs=1) as xgp:
                xgd = [
                    xgp.tile([128, C], BF16, name=f"xg{d}", tag=f"xg{d}")
                    for d in range(DC)
                ]
                xgr = xgT_in.ap().rearrange("p (a b) -> p a b", a=DC)
                for d in range(DC):
                    nc.sync.dma_start(xgd[d][:], xgr[:, d, :])
                # ---- mm1: hT[f, c] += w1[d, f].T-chunks @ xgT, then gelu ----
                with (
                    tc.tile_pool(name="w1bf", bufs=3) as w1bfp,
                    tc.tile_pool(name="ps1", bufs=2, space="PSUM") as ps1,
                ):
                    for f in range(FC):
                        w1b = w1bfp.tile([128, DC, 128], BF16)
                        nc.sync.dma_start(w1b[:], w1p.ap()[:, f, :, :])
                        ph = ps1.tile([128, C], F32)
                        for d in range(DC):
                            for h in range(0, C, NB):
                                nc.tensor.matmul(
                                    ph[:, h : h + NB],
                                    lhsT=w1b[:, d, :],
                                    rhs=xgd[d][:, h : h + NB],
                                    start=(d == 0),
                                    stop=(d == DC - 1),
                                )
                        # tanh-approx gelu (matches jax.nn.gelu default) in a
                        # single ScalarE pass, psum -> bf16 SBUF
                        nc.scalar.activation(
                            hT[:, f, :],
                            ph[:],
                            mybir.ActivationFunctionType.Gelu_apprx_tanh,
                        )

            # ---- mm2: outT[d, c] += w2[f, d-chunk].T @ hT[f, c] ----
            # (w2 stationary like mm1 -> one weight load per 1024 streamed
            #  columns; gate scale applied on the psum drain)
            with (
                tc.tile_pool(name="w2bf", bufs=3) as w2bfp,
                tc.tile_pool(name="outp", bufs=3) as outp,
                tc.tile_pool(name="ps2", bufs=2, space="PSUM") as ps2,
            ):
                for dch in range(DC):
                    pso = ps2.tile([128, C], F32)
                    for fg in range(FC // FG):
                        w2b = w2bfp.tile([128, FG, 128], BF16)
                        nc.sync.dma_start(
                            w2b[:], w2p.ap()[:, dch, fg * FG : (fg + 1) * FG, :]
                        )
                        for fi in range(FG):
                            f = fg * FG + fi
                            for h in range(0, C, NB):
                                nc.tensor.matmul(
                                    pso[:, h : h + NB],
                                    lhsT=w2b[:, fi, :],
                                    rhs=hT[:, f, h : h + NB],
                                    start=(f == 0),
                                    stop=(f == FC - 1),
                                )
                    ob = outp.tile([128, C], BF16)
                    nc.vector.tensor_mul(ob[:], pso[:], gbc[:])
                    nc.sync.dma_start(
                        out_cT.ap()[dch * 128 : (dch + 1) * 128, :], ob[:]
                    )
    nc.compile()
    return nc


# --------------------------------------------------------------------------
# Host orchestration
# --------------------------------------------------------------------------
@functools.lru_cache(maxsize=None)
def _compiled(S, D, E, F, n_cores):
    return (
        build_gate_nc(S, D, E, n_cores),
        build_route_nc(S, E),
        build_ffn_nc(S, D, E, F),
    )


def _run_spmd(nc, in_maps, **kw):
    res = run_bass_kernel_spmd(nc, in_maps, core_ids=list(range(len(in_maps))), **kw)
    return res.results


def _pack_w1(w1k, D, F):
    """w1 [D, F] f32 -> [128, FC, DC, 128] bf16 with
    w1p[p, f, c, j] = w1[c*128 + p, f*128 + j]."""
    DC, FC = D // 128, F // 128
    w = w1k.reshape(DC, 128, FC, 128).transpose(1, 2, 0, 3)
    return np.ascontiguousarray(w.astype(ml_dtypes.bfloat16))


def _pack_w2(w2k, F, D):
    """w2 [F, D] f32 -> [128, DC, FC, 128] bf16 with
    w2p[p, c, f, j] = w2[f*128 + p, c*128 + j]."""
    FC, DC = F // 128, D // 128
    w = w2k.reshape(FC, 128, DC, 128).transpose(1, 2, 0, 3)
    return np.ascontiguousarray(w.astype(ml_dtypes.bfloat16))


def moe_forward(hidden_states, w_gate, w1, w2, n_cores=N_CORES, run=_run_spmd):
    B, T, D = hidden_states.shape
    E = w_gate.shape[1]
    F = w1.shape[2]
    S = B * T
    C = S // E
    DC = D // 128
    Sl = S // n_cores
    J = S // 128
    x = np.ascontiguousarray(hidden_states.reshape(S, D), dtype=np.float32)
    nc_gate, nc_route, nc_ffn = _compiled(S, D, E, F, n_cores)

    # ---- launch 1: gate ----
    wg = np.ascontiguousarray(w_gate, dtype=np.float32)
    in1 = [
        {"xT": np.ascontiguousarray(x[k * Sl : (k + 1) * Sl].T), "wg": wg}
        for k in range(n_cores)
    ]
    res1 = run(nc_gate, in1)

    # per-core outputs concat: column j = (k, t) -> token s = j*128 + p
    ef = np.concatenate([r["e_out"] for r in res1], axis=1).astype(np.float32)
    gf = np.concatenate([r["g_out"] for r in res1], axis=1)  # [128, J] f32
    xb_full = np.concatenate(
        [x.astype(ml_dtypes.bfloat16), np.zeros((1, D), dtype=ml_dtypes.bfloat16)]
    )  # [S+1, D]

    # ---- launch 2: routing positions ----
    ut = np.tril(np.ones((128, 128), dtype=np.float32)).T  # ut[q, p] = q <= p
    in2 = [
        {"ef": ef, "eid": np.full((128, 1), float(k), np.float32), "ut": ut}
        for k in range(n_cores)
    ]
    res2 = run(nc_route, in2)

    # ---- host glue: build per-expert slot -> token index lists (placement) --
    s_val = (np.arange(J)[None, :] * 128 + np.arange(128)[:, None]).astype(
        np.int64
    )  # [128, J]
    g_vec = np.empty(S, dtype=np.float32)
    g_vec[s_val.reshape(-1)] = gf.reshape(-1)
    ids_all = []
    in3 = []
    for k in range(n_cores):
        ps_signed = res2[k]["pos_out"]
        valid = ps_signed > -0.5
        pos = np.maximum(ps_signed, 0.0)
        ids = np.full(C, S, dtype=np.int64)  # default -> zero row
        ids[pos[valid].astype(np.int64)] = s_val[valid]
        ids_all.append(ids)
        xg = xb_full[ids]  # [C, D] bf16
        xgT = np.ascontiguousarray(
            xg.T.reshape(DC, 128, C).transpose(1, 0, 2).reshape(128, DC * C)
        )
        g_slot = np.where(ids < S, g_vec[np.minimum(ids, S - 1)], 0.0).astype(
            np.float32
        )
        in3.append(
            {
                "xgT": xgT,
                "grow": np.ascontiguousarray(g_slot[None, :]),
                "w1p": _pack_w1(np.asarray(w1[k], dtype=np.float32), D, F),
                "w2p": _pack_w2(np.asarray(w2[k], dtype=np.float32), F, D),
            }
        )

    # ---- launch 3: FFN ----
    res3 = run(nc_ffn, in3)

    # ---- host scatter (placement only) ----
    out = np.zeros((S, D), dtype=np.float32)
    for k in range(n_cores):
        ids = ids_all[k]
        filled = ids < S
        out[ids[filled]] = res3[k]["out_cT"].T[filled].astype(np.float32)
    return out.reshape(B, T, D)


def kernel(**inputs):
    hs = np.asarray(inputs["hidden_states"], dtype=np.float32)
    wg = np.asarray(inputs["w_gate"], dtype=np.float32)
    w1 = np.asarray(inputs["w1"], dtype=np.float32)
    w2 = np.asarray(inputs["w2"], dtype=np.float32)
    return moe_forward(hs, wg, w1, w2)
